# revision 8
# baseline (speedup 1.0000x reference)
"""ALiBi multi-head attention on 8 trn2 NeuronCores (Bass/Tile).

Sharding: head+batch parallel. 16 heads x 2 batches = 32 (b,h) pairs; each of
the 8 cores owns 2 heads x 2 batches = 4 pairs. QKV/out projections are
tensor-parallel over heads (each core gets the 256 weight rows/cols for its
heads); the out-projection partial sums (over the head dimension) are reduced
on the host.

Per-core pipeline (all matmuls fp32r = single-pass reduced-precision fp32,
~1 cyc/col, measured 233 ns per 512-col matmul incl. LDWEIGHTS):
  A) stream xT tiles, produce Q^T,K^T (head-dim on partitions) and V ([j,d]
     via PE-transpose of V^T chunks).
  B) per (b,h) pair, per 512-col i-tile: S^T[j,i] = (K-chunk)^T Q matmuls;
     E = exp(S^T - slope*j) on ACT (ALiBi bias is per-partition in this
     layout because the slope*i term cancels in softmax_j; subtracting
     slope*i also stabilizes: max exponent ~ s_i0); causal mask via a
     precomputed triangle tile on the 4 diagonal chunks; denominator via
     ones-matmul (partition-dim reduction on PE, replicated across rows);
     normalize with DVE reciprocal+mult; one 3D DMA writes attn^T[j,i] per
     i-tile; PV accumulates attn_out^T[d,i] with V chunks as stationary.
  C) out-projection: lhsT = attn_out^T chunks, rhs = Wo^T slice; partial
     (4096, 2048) written per core, summed on host.
"""

import sys

sys.path.insert(0, "/opt/trn_rl_repo")

import numpy as np

import concourse.bass as bass
import concourse.mybir as mybir
import concourse.tile as tile
from concourse import bacc
from concourse.bass_utils import run_bass_kernel_spmd
from concourse.masks import make_identity

B, T, C, H = 2, 2048, 2048, 16
HD = C // H  # 128
NCORES = 8
HPC = H // NCORES  # 2 heads per core
TOK = B * T  # 4096
KO = C // 128  # 16 contraction chunks
NT = T // 512  # 4 i-tiles per (b,h)
TOKT = TOK // 512  # 8 token tiles
F32 = mybir.dt.float32
F32R = mybir.dt.float32r
EXPF = mybir.ActivationFunctionType.Exp
MULT = mybir.AluOpType.mult

_cached = {}


def _build():
    nc = bacc.Bacc(None, target_bir_lowering=False)

    xT_d = nc.dram_tensor("xT", [KO, 128, TOK], F32, kind="ExternalInput")
    wq_d = nc.dram_tensor("wqT", [KO, 128, 256], F32, kind="ExternalInput")
    wk_d = nc.dram_tensor("wkT", [KO, 128, 256], F32, kind="ExternalInput")
    wv_d = nc.dram_tensor("wvT", [KO, 128, 256], F32, kind="ExternalInput")
    wo_d = nc.dram_tensor("woT", [HPC, 128, C], F32, kind="ExternalInput")
    cb_d = nc.dram_tensor("colbias", [128, HPC * KO], F32, kind="ExternalInput")
    attn_d = nc.dram_tensor("attnT", [2 * HPC, T, T], F32, kind="ExternalOutput")
    out_d = nc.dram_tensor("outp", [TOK, C], F32, kind="ExternalOutput")

    def r(ap):
        return ap.bitcast(F32R)

    with tile.TileContext(nc) as tc:
        with (
            tc.tile_pool(name="resident", bufs=1) as rp,
            tc.tile_pool(name="consts", bufs=1) as cp,
        ):
            # constants
            # wide causal masks: mk[o][p, q] = 1 if q >= 128*o + p else 0
            mks = []
            for o in range(4):
                mk = cp.tile([128, 512], F32, name=f"mk{o}")
                nc.gpsimd.memset(mk[:], 1.0)
                nc.gpsimd.affine_select(
                    out=mk[:], in_=mk[:], compare_op=mybir.AluOpType.is_ge,
                    fill=0.0, base=-128 * o, pattern=[[1, 512]], channel_multiplier=-1,
                )
                mks.append(mk)
            ident = cp.tile([128, 128], F32)
            make_identity(nc, ident[:])
            ones_dram = nc.inline_tensor(np.ones((128, 128), np.float32), name="ones_c")
            ones = cp.tile([128, 128], F32R)
            nc.sync.dma_start(ones[:], ones_dram[:].bitcast(F32R))
            cb = cp.tile([128, HPC * KO], F32)
            nc.sync.dma_start(cb[:], cb_d[:])

            # residents
            QT = rp.tile([128, 2 * HPC, T], F32)  # [d, slot, tok_local]
            KT = rp.tile([128, 2 * HPC, T], F32)
            V = rp.tile([128, B * KO, 256], F32)  # [j, (b,jc), d_2heads]

            # ---------------- Phase A: projections ----------------
            with (
                tc.tile_pool(name="wpool", bufs=1) as wp,
                tc.tile_pool(name="xt", bufs=3) as xtp,
                tc.tile_pool(name="vt", bufs=2) as vtp,
                tc.tile_pool(name="pa", bufs=1, space="PSUM") as pa,
                tc.tile_pool(name="tp", bufs=2, space="PSUM") as tpp,
            ):
                wq = wp.tile([128, KO, 256], F32R)
                wk = wp.tile([128, KO, 256], F32R)
                wv = wp.tile([128, KO, 256], F32R)
                nc.sync.dma_start(wq[:], r(wq_d[:].rearrange("ko ki d -> ki ko d")))
                nc.sync.dma_start(wk[:], r(wk_d[:].rearrange("ko ki d -> ki ko d")))
                nc.sync.dma_start(wv[:], r(wv_d[:].rearrange("ko ki d -> ki ko d")))

                for tt in range(TOKT):
                    b = tt // (TOKT // B)
                    tloc = tt % (TOKT // B)
                    halves = []
                    for hh in range(2):
                        xh = xtp.tile([128, KO // 2, 512], F32R, tag="xt")
                        nc.sync.dma_start(
                            xh[:],
                            r(
                                xT_d[hh * 8 : hh * 8 + 8, :, tt * 512 : tt * 512 + 512]
                                .rearrange("ko ki t -> ki ko t")
                            ),
                        )
                        halves.append(xh)
                    ps = {}
                    for j, nm in enumerate(("q0", "q1", "k0", "k1", "v0", "v1")):
                        ps[nm] = pa.tile([128, 512], F32, tag=f"pp{j}", name=f"pp{j}")
                    for ko in range(KO):
                        xs = halves[ko // 8][:, ko % 8, :]
                        st = ko == 0
                        sp = ko == KO - 1
                        for h in range(2):
                            ds = slice(h * 128, h * 128 + 128)
                            nc.tensor.matmul(ps[f"q{h}"][:], wq[:, ko, ds], xs, start=st, stop=sp)
                            nc.tensor.matmul(ps[f"k{h}"][:], wk[:, ko, ds], xs, start=st, stop=sp)
                            nc.tensor.matmul(ps[f"v{h}"][:], wv[:, ko, ds], xs, start=st, stop=sp)
                    tsl = slice(tloc * 512, tloc * 512 + 512)
                    for h in range(2):
                        slot = 2 * h + b
                        nc.scalar.copy(r(QT[:, slot, tsl]), ps[f"q{h}"][:])
                        nc.scalar.copy(r(KT[:, slot, tsl]), ps[f"k{h}"][:])
                        vt = vtp.tile([128, 512], F32, tag="vt")
                        nc.scalar.copy(vt[:], ps[f"v{h}"][:])
                        for q in range(4):
                            jc = tloc * 4 + q
                            tp = tpp.tile([128, 128], F32, tag="tp")
                            nc.tensor.transpose(tp[:], vt[:, q * 128 : q * 128 + 128], ident[:])
                            nc.vector.tensor_copy(
                                out=r(V[:, b * KO + jc, h * 128 : h * 128 + 128]),
                                in_=tp[:],
                            )

            # ---------------- Phase B: attention ----------------
            # aoT lives in its own pool opened after phase A's pools close,
            # so phase A can use the space
            attn_r = attn_d[:].rearrange("s (jc p) i -> s p jc i", p=128)
            aop_cm = tc.tile_pool(name="ao", bufs=1)
            aop = aop_cm.__enter__()
            with (
                tc.tile_pool(name="exp", bufs=2) as expp,
                tc.tile_pool(name="sbb", bufs=2) as sbb,
                tc.tile_pool(name="pss", bufs=2, space="PSUM") as pss,
                tc.tile_pool(name="psd", bufs=2, space="PSUM") as psd,
                tc.tile_pool(name="psa", bufs=2, space="PSUM") as psa,
            ):
                aoT = aop.tile([128, 2 * HPC, T], F32)
                for slot in range(2 * HPC):
                    b = slot % 2
                    h = slot // 2
                    for t in range(NT):
                        J = 4 * (t + 1)
                        eb = expp.tile([128, KO, 512], F32, tag="exp")
                        den = psd.tile([128, 512], F32, tag="den")
                        isl = slice(t * 512, t * 512 + 512)
                        for jc in range(J):
                            sp = pss.tile([128, 512], F32, tag="s")
                            nc.tensor.matmul(
                                sp[:],
                                r(KT[:, slot, jc * 128 : jc * 128 + 128]),
                                r(QT[:, slot, isl]),
                                start=True, stop=True,
                            )
                            bias = cb[:, h * KO + jc : h * KO + jc + 1]
                            off = 128 * jc - 512 * t
                            nc.scalar.activation(r(eb[:, jc, :]), sp[:], EXPF, bias=bias)
                            if off >= 0:  # diagonal chunk: causal mask
                                nc.vector.tensor_mul(
                                    r(eb[:, jc]), eb[:, jc], mks[off // 128][:]
                                )
                            nc.tensor.matmul(
                                den[:], ones[:], r(eb[:, jc]),
                                start=(jc == 0), stop=(jc == J - 1),
                            )
                        rcp = sbb.tile([128, 512], F32, tag="rcp")
                        nc.vector.reciprocal(rcp[:], den[:])
                        for jc in range(J):
                            nc.vector.tensor_mul(r(eb[:, jc]), eb[:, jc], rcp[:])
                        nc.sync.dma_start(attn_r[slot, :, 0:J, isl], eb[:, 0:J, :])
                        ao = psa.tile([128, 512], F32, tag="ao")
                        for jc in range(J):
                            nc.tensor.matmul(
                                ao[:],
                                r(V[:, b * KO + jc, h * 128 : h * 128 + 128]),
                                r(eb[:, jc]),
                                start=(jc == 0), stop=(jc == J - 1),
                            )
                        nc.scalar.copy(r(aoT[:, slot, isl]), ao[:])

            # ---------------- Phase C: out projection ----------------
            with (
                tc.tile_pool(name="wo", bufs=1) as wop,
                tc.tile_pool(name="osb", bufs=3) as osb,
                tc.tile_pool(name="pso", bufs=4, space="PSUM") as pso,
            ):
                wo = wop.tile([128, HPC, C], F32R)
                nc.sync.dma_start(wo[:], r(wo_d[:].rearrange("h ki c -> ki h c")))
                for b in range(B):
                    for ic in range(T // 128):
                        ob = osb.tile([128, C], F32, tag="ob")
                        csl = slice(ic * 128, ic * 128 + 128)
                        for ct in range(C // 512):
                            op = pso.tile([128, 512], F32, tag="o")
                            for h in range(HPC):
                                nc.tensor.matmul(
                                    op[:],
                                    r(aoT[:, 2 * h + b, csl]),
                                    wo[:, h, ct * 512 : ct * 512 + 512],
                                    start=(h == 0), stop=(h == HPC - 1),
                                )
                            nc.scalar.copy(ob[:, ct * 512 : ct * 512 + 512], op[:])
                        row0 = b * T + ic * 128
                        nc.sync.dma_start(out_d[row0 : row0 + 128, :], ob[:])
            aop_cm.__exit__(None, None, None)

    nc.compile()
    return nc


def _get_nc():
    if "nc" not in _cached:
        _cached["nc"] = _build()
    return _cached["nc"]


def kernel(x, Wq, Wk, Wv, Wo, bo):
    x = np.asarray(x, np.float32)
    Wq = np.asarray(Wq, np.float32)
    Wk = np.asarray(Wk, np.float32)
    Wv = np.asarray(Wv, np.float32)
    Wo = np.asarray(Wo, np.float32)
    bo = np.asarray(bo, np.float32)

    nc = _get_nc()
    scale = 1.0 / np.sqrt(HD)
    xTr = np.ascontiguousarray(x.reshape(TOK, C).T).reshape(KO, 128, TOK)
    slopes = np.array([1.0 / 2 ** (i + 1) for i in range(H)], np.float32)

    in_maps = []
    for c in range(NCORES):
        rows = slice(256 * c, 256 * c + 256)
        wqT = np.ascontiguousarray((Wq[rows] * scale).T).reshape(KO, 128, 256)
        wkT = np.ascontiguousarray(Wk[rows].T).reshape(KO, 128, 256)
        wvT = np.ascontiguousarray(Wv[rows].T).reshape(KO, 128, 256)
        woT = np.ascontiguousarray(Wo[:, rows].T).reshape(HPC, 128, C)
        p = np.arange(128, dtype=np.float32)[:, None]
        jcs = np.arange(KO, dtype=np.float32)[None, :]
        cbs = []
        for h in range(HPC):
            s = slopes[HPC * c + h]
            cbs.append(-s * (128.0 * jcs + p))  # [128, KO]
        colbias = np.concatenate(cbs, axis=1).astype(np.float32)
        in_maps.append(
            {"xT": xTr, "wqT": wqT, "wkT": wkT, "wvT": wvT, "woT": woT,
             "colbias": colbias}
        )

    res = run_bass_kernel_spmd(nc, in_maps, core_ids=list(range(NCORES)))

    out = np.zeros((TOK, C), np.float32)
    attn = np.empty((B, H, T, T), np.float32)
    for c in range(NCORES):
        out += res.results[c]["outp"]
        at = res.results[c]["attnT"]
        for slot in range(2 * HPC):
            b = slot % 2
            h = HPC * c + slot // 2
            attn[b, h] = at[slot].T
    out = (out + bo[None, :]).reshape(B, T, C)
    return out, attn


if __name__ == "__main__":
    rng = np.random.default_rng(0)
    x = rng.standard_normal((B, T, C)).astype(np.float32)
    Wq = (rng.standard_normal((C, C)) * 0.02).astype(np.float32)
    Wk = (rng.standard_normal((C, C)) * 0.02).astype(np.float32)
    Wv = (rng.standard_normal((C, C)) * 0.02).astype(np.float32)
    Wo = (rng.standard_normal((C, C)) * 0.02).astype(np.float32)
    bo = np.zeros((C,), np.float32)
    out, attn = kernel(x, Wq, Wk, Wv, Wo, bo)
    print("out", out.shape, out.dtype, "attn", attn.shape, attn.dtype)


# revision 13
# speedup vs baseline: 1.1043x; 1.1043x over previous
"""ALiBi multi-head attention on 8 trn2 NeuronCores (Bass/Tile).

Sharding: head+batch parallel. 16 heads x 2 batches = 32 (b,h) pairs; each of
the 8 cores owns 2 heads x 2 batches = 4 pairs (tensor-parallel projections
over heads; out-projection partials summed on the host). Per batch: stream
xT, produce Q^T/K^T (head-dim on partitions) and V (PE-transposed), then
flash-style attention in the transposed layout — S^T[j,i] matmuls, ACT exp
with the ALiBi bias folded into the per-partition bias operand (the slope*i
term cancels in softmax_j and doubles as the stabilizer), ones-matmul
denominator, PV on unnormalized exp tiles, post-PV normalization split
DVE/GPSIMD, reciprocal via ACT ln->exp, out-projection interleaved per
i-tile. All matmuls fp32r (~1 cyc/col, 1.5e-4 matmul precision).
"""

import sys

sys.path.insert(0, "/opt/trn_rl_repo")

import numpy as np

import concourse.mybir as mybir
import concourse.tile as tile
from concourse import bacc
from concourse.bass_utils import run_bass_kernel_spmd
from concourse.masks import make_identity

B, T, C, H = 2, 2048, 2048, 16
HD = C // H
NCORES = 8
HPC = H // NCORES
TOK = B * T
KO = C // 128
NT = T // 512
F32 = mybir.dt.float32
F32R = mybir.dt.float32r
EXPF = mybir.ActivationFunctionType.Exp
MULT = mybir.AluOpType.mult
LAG = 3

_cached = {}


def _build():
    nc = bacc.Bacc(None, target_bir_lowering=False)

    xT_d = nc.dram_tensor("xT", [KO, 128, TOK], F32, kind="ExternalInput")
    wq_d = nc.dram_tensor("wqT", [KO, 128, 256], F32, kind="ExternalInput")
    wk_d = nc.dram_tensor("wkT", [KO, 128, 256], F32, kind="ExternalInput")
    wv_d = nc.dram_tensor("wvT", [KO, 128, 256], F32, kind="ExternalInput")
    wo_d = nc.dram_tensor("woT", [HPC, 128, C], F32, kind="ExternalInput")
    cb_d = nc.dram_tensor("colbias", [128, HPC * KO], F32, kind="ExternalInput")
    attn_d = nc.dram_tensor("attnT", [2 * HPC, T, T], F32, kind="ExternalOutput")
    out_d = nc.dram_tensor("outp", [TOK, C], F32, kind="ExternalOutput")

    def r(ap):
        return ap.bitcast(F32R)

    with tile.TileContext(nc) as tc:
        with tc.tile_pool(name="consts", bufs=1) as cp:
            tri = cp.tile([128, 128], F32)  # tri[p,q] = 1 if q >= p else 0
            nc.gpsimd.memset(tri[:], 1.0)
            nc.gpsimd.affine_select(
                out=tri[:], in_=tri[:], compare_op=mybir.AluOpType.is_ge,
                fill=0.0, base=0, pattern=[[1, 128]], channel_multiplier=-1,
            )
            ident = cp.tile([128, 128], F32)
            make_identity(nc, ident[:])
            ones_dram = nc.inline_tensor(np.ones((128, 128), np.float32), name="ones_c")
            ones = cp.tile([128, 128], F32R)
            nc.sync.dma_start(ones[:], ones_dram[:].bitcast(F32R))
            cb = cp.tile([128, HPC * KO], F32)
            nc.sync.dma_start(cb[:], cb_d[:])
            wo = cp.tile([128, HPC, C], F32R)
            nc.sync.dma_start(wo[:], r(wo_d[:].rearrange("h ki c -> ki h c")))

            for b in range(B):
                with tc.tile_pool(name=f"qkv{b}", bufs=1) as rp:
                    QT = rp.tile([128, HPC, T], F32, name="QT")
                    KT = rp.tile([128, HPC, T], F32, name="KT")
                    V = rp.tile([128, KO, 256], F32, name="V")
                    aoT = rp.tile([128, HPC, T], F32, name="aoT")

                    # ---- Phase A(b): projections for this batch ----
                    with (
                        tc.tile_pool(name=f"wp{b}", bufs=1) as wp,
                        tc.tile_pool(name=f"xt{b}", bufs=3) as xtp,
                        tc.tile_pool(name=f"vt{b}", bufs=2) as vtp,
                        tc.tile_pool(name=f"pa{b}", bufs=1, space="PSUM") as pa,
                        tc.tile_pool(name=f"tp{b}", bufs=2, space="PSUM") as tpp,
                    ):
                        wq = wp.tile([128, KO, 256], F32R, name="wq")
                        wk = wp.tile([128, KO, 256], F32R, name="wk")
                        wv = wp.tile([128, KO, 256], F32R, name="wv")
                        nc.sync.dma_start(wq[:], r(wq_d[:].rearrange("ko ki d -> ki ko d")))
                        nc.sync.dma_start(wk[:], r(wk_d[:].rearrange("ko ki d -> ki ko d")))
                        nc.sync.dma_start(wv[:], r(wv_d[:].rearrange("ko ki d -> ki ko d")))

                        for tloc in range(NT):
                            tg = b * T + tloc * 512  # global token offset
                            halves = []
                            for hh in range(2):
                                xh = xtp.tile([128, KO // 2, 512], F32R, tag="xt", name="xh")
                                nc.sync.dma_start(
                                    xh[:],
                                    r(
                                        xT_d[hh * 8 : hh * 8 + 8, :, tg : tg + 512]
                                        .rearrange("ko ki t -> ki ko t")
                                    ),
                                )
                                halves.append(xh)
                            ps = {}
                            for j, nm in enumerate(("q0", "q1", "k0", "k1", "v0", "v1")):
                                ps[nm] = pa.tile([128, 512], F32, tag=f"pp{j}", name=f"pp{j}")
                            for ko in range(KO):
                                xs = halves[ko // 8][:, ko % 8, :]
                                st = ko == 0
                                sp = ko == KO - 1
                                for h in range(HPC):
                                    ds = slice(h * 128, h * 128 + 128)
                                    nc.tensor.matmul(ps[f"q{h}"][:], wq[:, ko, ds], xs, start=st, stop=sp)
                                    nc.tensor.matmul(ps[f"k{h}"][:], wk[:, ko, ds], xs, start=st, stop=sp)
                                    nc.tensor.matmul(ps[f"v{h}"][:], wv[:, ko, ds], xs, start=st, stop=sp)
                            tsl = slice(tloc * 512, tloc * 512 + 512)
                            for h in range(HPC):
                                nc.scalar.copy(r(QT[:, h, tsl]), ps[f"q{h}"][:])
                                nc.scalar.copy(r(KT[:, h, tsl]), ps[f"k{h}"][:])
                                vt = vtp.tile([128, 512], F32, tag="vt", name="vt")
                                nc.scalar.copy(vt[:], ps[f"v{h}"][:])
                                for q in range(4):
                                    jc = tloc * 4 + q
                                    tp = tpp.tile([128, 128], F32, tag="tp", name="tp")
                                    nc.tensor.transpose(tp[:], vt[:, q * 128 : q * 128 + 128], ident[:])
                                    nc.vector.tensor_copy(
                                        out=r(V[:, jc, h * 128 : h * 128 + 128]),
                                        in_=tp[:],
                                    )

                    # ---- Phases B+C(b): attention + out-projection ----
                    # PV consumes UNNORMALIZED exp tiles so the PE stream
                    # (S^T, den, PV, out-proj) never waits on the softmax
                    # division; den/PV lag the S^T stream by LAG chunks;
                    # attn-tile normalization is post-PV (plain f32, split
                    # DVE/GPSIMD); attn_out^T normalized once per i-tile from
                    # PSUM; out-projection emitted per (b, i-tile).
                    attn_r = attn_d[:].rearrange("s (jc p) i -> s p jc i", p=128)
                    with (
                        tc.tile_pool(name=f"exp{b}", bufs=2) as expp,
                        tc.tile_pool(name=f"sbb{b}", bufs=2) as sbb,
                        tc.tile_pool(name=f"stg{b}", bufs=8) as stp,
                        tc.tile_pool(name=f"osb{b}", bufs=3) as osb,
                        tc.tile_pool(name=f"pss{b}", bufs=3, space="PSUM") as pss,
                        tc.tile_pool(name=f"psd{b}", bufs=2, space="PSUM") as psd,
                        tc.tile_pool(name=f"psa{b}", bufs=1, space="PSUM") as psa,
                        tc.tile_pool(name=f"pso{b}", bufs=2, space="PSUM") as pso,
                    ):
                        for t in range(NT):
                            J = 4 * (t + 1)
                            isl = slice(t * 512, t * 512 + 512)
                            for h in range(HPC):
                                slot = 2 * h + b
                                eb = expp.tile([128, KO, 512], F32, tag="exp", name="eb")
                                den = psd.tile([128, 512], F32, tag="den", name="den")
                                ao = psa.tile([128, 512], F32, tag="ao", name="ao")

                                def rng(jc):
                                    return slice(max(128 * jc - 512 * t, 0), 512)

                                def den_pv(jc):
                                    nc.tensor.matmul(
                                        den[:, rng(jc)], ones[:], r(eb[:, jc, rng(jc)]),
                                        start=(jc == 0), stop=(jc == J - 1),
                                        skip_group_check=True,
                                    )
                                    nc.tensor.matmul(
                                        ao[:, rng(jc)],
                                        r(V[:, jc, h * 128 : h * 128 + 128]),
                                        r(eb[:, jc, rng(jc)]),
                                        start=(jc == 0), stop=(jc == J - 1),
                                        skip_group_check=True,
                                    )

                                for jc in range(J):
                                    sp = pss.tile([128, 512], F32, tag="s", name="sp")
                                    nc.tensor.matmul(
                                        sp[:],
                                        r(KT[:, h, jc * 128 : jc * 128 + 128]),
                                        r(QT[:, h, isl]),
                                        start=True, stop=True,
                                    )
                                    bias = cb[:, h * KO + jc : h * KO + jc + 1]
                                    off = 128 * jc - 512 * t
                                    nc.scalar.activation(
                                        r(eb[:, jc, rng(jc)]), sp[:, rng(jc)], EXPF,
                                        bias=bias,
                                    )
                                    if off >= 0:  # causal triangle on diagonal
                                        nc.vector.tensor_mul(
                                            r(eb[:, jc, off : off + 128]),
                                            eb[:, jc, off : off + 128],
                                            tri[:],
                                        )
                                    if jc >= LAG:
                                        den_pv(jc - LAG)
                                for jc in range(max(J - LAG, 0), J):
                                    den_pv(jc)
                                # reciprocal via ACT ln -> exp(-x)
                                lnt = sbb.tile([128, 512], F32, tag="ln", name="lnt")
                                rcp = sbb.tile([128, 512], F32, tag="rcp", name="rcp")
                                nc.scalar.activation(
                                    lnt[:], den[:], mybir.ActivationFunctionType.Ln
                                )
                                nc.scalar.activation(rcp[:], lnt[:], EXPF, scale=-1.0)
                                nc.vector.tensor_tensor(
                                    r(aoT[:, h, isl]), ao[:], rcp[:], MULT
                                )
                                # normalize into staging tiles (plain f32 —
                                # GPSIMD-writable, never aliased with f32r
                                # matmul reads) and DMA each to attn output
                                for jc in range(J):
                                    eng = nc.vector if jc % 2 == 0 else nc.gpsimd
                                    st = stp.tile([128, 512], F32, tag="st", name="st")
                                    g = rng(jc)
                                    eng.tensor_mul(st[:, g], eb[:, jc, g], rcp[:, g])
                                    off = max(128 * jc - 512 * t, 0)
                                    nc.sync.dma_start(
                                        attn_r[slot, :, jc, t * 512 + off : (t + 1) * 512],
                                        st[:, g],
                                    )
                            # out-projection for this batch's i-chunks
                            for ic in range(4 * t, 4 * t + 4):
                                ob = osb.tile([128, C], F32, tag="ob", name="ob")
                                csl = slice(ic * 128, ic * 128 + 128)
                                for ct in range(C // 512):
                                    op = pso.tile([128, 512], F32, tag="o", name="op")
                                    for h in range(HPC):
                                        nc.tensor.matmul(
                                            op[:],
                                            r(aoT[:, h, csl]),
                                            wo[:, h, ct * 512 : ct * 512 + 512],
                                            start=(h == 0), stop=(h == HPC - 1),
                                        )
                                    if ct % 2 == 0:
                                        nc.scalar.copy(ob[:, ct * 512 : ct * 512 + 512], op[:])
                                    else:
                                        nc.vector.tensor_copy(
                                            out=ob[:, ct * 512 : ct * 512 + 512], in_=op[:]
                                        )
                                row0 = b * T + ic * 128
                                nc.sync.dma_start(out_d[row0 : row0 + 128, :], ob[:])

    nc.compile()
    return nc


def _get_nc():
    if "nc" not in _cached:
        _cached["nc"] = _build()
    return _cached["nc"]


def kernel(x, Wq, Wk, Wv, Wo, bo):
    x = np.asarray(x, np.float32)
    Wq = np.asarray(Wq, np.float32)
    Wk = np.asarray(Wk, np.float32)
    Wv = np.asarray(Wv, np.float32)
    Wo = np.asarray(Wo, np.float32)
    bo = np.asarray(bo, np.float32)

    nc = _get_nc()
    scale = 1.0 / np.sqrt(HD)
    xTr = np.ascontiguousarray(x.reshape(TOK, C).T).reshape(KO, 128, TOK)
    slopes = np.array([1.0 / 2 ** (i + 1) for i in range(H)], np.float32)

    in_maps = []
    for c in range(NCORES):
        rows = slice(256 * c, 256 * c + 256)
        wqT = np.ascontiguousarray((Wq[rows] * scale).T).reshape(KO, 128, 256)
        wkT = np.ascontiguousarray(Wk[rows].T).reshape(KO, 128, 256)
        wvT = np.ascontiguousarray(Wv[rows].T).reshape(KO, 128, 256)
        woT = np.ascontiguousarray(Wo[:, rows].T).reshape(HPC, 128, C)
        p = np.arange(128, dtype=np.float32)[:, None]
        jcs = np.arange(KO, dtype=np.float32)[None, :]
        cbs = []
        for h in range(HPC):
            s = slopes[HPC * c + h]
            cbs.append(-s * (128.0 * jcs + p))  # [128, KO]
        colbias = np.concatenate(cbs, axis=1).astype(np.float32)
        in_maps.append(
            {"xT": xTr, "wqT": wqT, "wkT": wkT, "wvT": wvT, "woT": woT,
             "colbias": colbias}
        )

    res = run_bass_kernel_spmd(nc, in_maps, core_ids=list(range(NCORES)))

    out = np.zeros((TOK, C), np.float32)
    attn = np.empty((B, H, T, T), np.float32)
    for c in range(NCORES):
        out += res.results[c]["outp"]
        at = res.results[c]["attnT"]
        for slot in range(2 * HPC):
            b = slot % 2
            h = HPC * c + slot // 2
            attn[b, h] = at[slot].T
    out = (out + bo[None, :]).reshape(B, T, C)
    return out, attn


if __name__ == "__main__":
    rng = np.random.default_rng(0)
    x = rng.standard_normal((B, T, C)).astype(np.float32)
    Wq = (rng.standard_normal((C, C)) * 0.02).astype(np.float32)
    Wk = (rng.standard_normal((C, C)) * 0.02).astype(np.float32)
    Wv = (rng.standard_normal((C, C)) * 0.02).astype(np.float32)
    Wo = (rng.standard_normal((C, C)) * 0.02).astype(np.float32)
    bo = np.zeros((C,), np.float32)
    out, attn = kernel(x, Wq, Wk, Wv, Wo, bo)
    print("out", out.shape, out.dtype, "attn", attn.shape, attn.dtype)
